# revision 2
# baseline (speedup 1.0000x reference)
"""Masked multi-head attention on 8 Trainium2 NeuronCores (Bass/Tile).

Problem: Q,K,V [2, 16, 2048, 64] f32, mask [2, 1, 2048, 2048] bool ->
softmax(where(mask, -inf, QK^T) / sqrt(64)) @ V, computed as one SPMD Bass
program over 8 cores; each core owns 4 heads of one batch ((B,H) sharding).

Per-core kernel (per head, per 512-wide q-chunk), all bf16 matmul inputs:
  - scores^T[k, q] = K^T Q: bf16 matmuls, the D=64 contraction row-packed two
    k-blocks at a time into PE row groups (0,0)/(64,0).
  - mask: additive -240 bias in fp8e4m3 ([k, q]-transposed, host-prepared).
    One of 8 k-block groups per q-chunk gets it via a PE identity-matmul
    preload into PSUM (QK accumulates on top); the rest via DVE tensor_add in
    PSUM. This split balances DVE vs PE occupancy (measured, not modeled:
    more PE groups poison the QK accumulation pipeline).
  - exp on the Scalar engine with scale=1/8, PSUM -> SBUF, bf16 slab out.
    No max-subtraction: unmasked logits are O(5) and softmax is
    shift-invariant; masked logits sit at -30 after scaling (exp ~ 2e-11).
  - O^T = V_aug^T @ P^T bf16 matmuls accumulating over k in PSUM; V_aug has a
    ones column appended so row 64 of the accumulator is the softmax
    denominator for free.
  - Scalar-engine copy PSUM->SBUF (keeps DVE off the critical tail), PE
    transposes [65 x 128] tiles back to [q, d], one narrow DVE reciprocal
    (free dim 4 - wide reciprocals are ~7 cyc/elem) and one broadcasted DVE
    multiply normalize, DMA out (p-major layout, unshuffled on the host).
"""

import sys

sys.path.insert(0, "/opt/trn_rl_repo")

from contextlib import ExitStack

import numpy as np
import ml_dtypes

N_CORES = 8
B, HFULL, S, D = 2, 16, 2048, 64
H = (B * HFULL) // N_CORES  # heads per core
QC = 512
KB = S // 128
NQC = S // QC
NJ = QC // 128
G = 2  # k-blocks per PSUM scores tile
NG = KB // G
MASK_NEG = -240.0  # max finite in fp8e4m3 (IEEE variant); *1/8 -> -30 logit shift
MASK_ON_PE = 0.125  # fraction of k-block groups whose mask-add runs on PE

_STATE = {}


def _build_program():
    import concourse.bass as bass
    import concourse.tile as tile
    from concourse import bacc, mybir
    from concourse.masks import make_identity

    F32 = mybir.dt.float32
    BF16 = mybir.dt.bfloat16
    FP8 = mybir.dt.float8e4

    nc = bacc.Bacc(
        "TRN2", target_bir_lowering=False, debug=False, enable_partition_id=False
    )

    qkt = nc.dram_tensor("qkt", [H, 64, 2, S], BF16, kind="ExternalInput").ap()
    vaug = nc.dram_tensor("vaug", [H, KB, 128, D + 1], BF16, kind="ExternalInput").ap()
    mb = nc.dram_tensor("mb", [128, NG, G, S], FP8, kind="ExternalInput").ap()
    out = nc.dram_tensor("out", [H, NQC, 128, NJ, D], F32, kind="ExternalOutput").ap()

    n_pe_mask = int(round(NG * MASK_ON_PE))

    with tile.TileContext(nc) as tc, ExitStack() as ctx:
        const_pool = ctx.enter_context(tc.tile_pool(name="const", bufs=1))
        mb_pool = ctx.enter_context(tc.tile_pool(name="mbp", bufs=1))
        qk_pool = ctx.enter_context(tc.tile_pool(name="qkp", bufs=2))
        v_pool = ctx.enter_context(tc.tile_pool(name="vp", bufs=2))
        slab_pool = ctx.enter_context(tc.tile_pool(name="slab", bufs=2 * NG))
        o_pool = ctx.enter_context(tc.tile_pool(name="op", bufs=2))
        small_pool = ctx.enter_context(tc.tile_pool(name="smallp", bufs=8))
        ps_score_pool = ctx.enter_context(
            tc.tile_pool(name="psscore", bufs=3, space="PSUM")
        )
        ps_o_pool = ctx.enter_context(tc.tile_pool(name="pso", bufs=1, space="PSUM"))
        ps_t_pool = ctx.enter_context(tc.tile_pool(name="pst", bufs=1, space="PSUM"))

        ident_f = const_pool.tile([128, 128], F32)
        make_identity(nc, ident_f)
        ident_b = const_pool.tile([128, 128], FP8)
        make_identity(nc, ident_b)

        mb_t = mb_pool.tile([128, NG, G, S], FP8, tag="mb")
        hsplit = max(1, NG // 4)
        nc.sync.dma_start(mb_t[:, :hsplit], mb[:, :hsplit])
        nc.sync.dma_start(mb_t[:, hsplit:], mb[:, hsplit:])

        for h in range(H):
            qk_t = qk_pool.tile([128, 2, S], BF16, tag="qk")
            nc.sync.dma_start(qk_t[:64, :, :], qkt[h])
            nc.sync.dma_start(qk_t[64:, :, :], qkt[h])
            qt_t = qk_t[:, 0, :]
            kt_t = qk_t[:, 1, :]
            v_t = v_pool.tile([128, KB, D + 1], BF16, tag="v")
            nc.gpsimd.dma_start(v_t[:], vaug[h].rearrange("kb p d -> p kb d"))

            for qc in range(NQC):
                qsl = bass.ts(qc, QC)
                slabs = []
                for g in range(NG):
                    ps = ps_score_pool.tile([128, G, QC], F32, tag="psscore")
                    on_pe = n_pe_mask > 0 and g % (NG // n_pe_mask) == 0
                    for i in range(G):
                        kb = G * g + i
                        half = kb % 2
                        lo, hi = half * 64, half * 64 + 64
                        psl = ps[:, i, :]
                        if on_pe:
                            nc.tensor.matmul(
                                psl,
                                ident_b[:],
                                mb_t[:, g, i, qsl],
                                start=True,
                                stop=False,
                            )
                        nc.tensor.matmul(
                            psl,
                            kt_t[lo:hi, bass.ts(kb, 128)],
                            qt_t[lo:hi, qsl],
                            start=not on_pe,
                            stop=True,
                        )
                    if not on_pe:
                        nc.vector.tensor_add(ps[:], ps[:], mb_t[:, g, :, qsl])
                    slab = slab_pool.tile([128, G, QC], BF16, tag="slab")
                    nc.scalar.activation(
                        slab[:], ps[:], mybir.ActivationFunctionType.Exp, scale=0.125
                    )
                    slabs.append(slab)

                ps_o = ps_o_pool.tile([D + 1, QC], F32, tag="pso")
                for kb in range(KB):
                    nc.tensor.matmul(
                        ps_o[:],
                        v_t[:, kb, :],
                        slabs[kb // G][:, kb % G, :],
                        start=(kb == 0),
                        stop=(kb == KB - 1),
                    )
                o_sb = o_pool.tile([D + 1, QC], F32, tag="osb")
                nc.scalar.copy(o_sb[:], ps_o[:])

                out_sb = o_pool.tile([128, NJ, D], F32, tag="outsb")
                ps_t = ps_t_pool.tile([128, NJ, D + 1], F32, tag="pst")
                for j in range(NJ):
                    nc.tensor.transpose(
                        ps_t[:, j, :],
                        o_sb[:, bass.ts(j, 128)],
                        ident_f[: D + 1, : D + 1],
                    )
                rcp = small_pool.tile([128, NJ], F32, tag="rcp")
                nc.vector.reciprocal(rcp[:], ps_t[:, :, D])
                nc.vector.tensor_mul(
                    out_sb[:], ps_t[:, :, :D], rcp[:].broadcast_to((128, NJ, D))
                )
                nc.gpsimd.dma_start(out[h, qc], out_sb[:])

    nc.compile()
    return nc


class _Runner:
    """shard_map jit over the 8 NeuronCores, reusable across calls."""

    def __init__(self, nc):
        import jax
        from jax.sharding import Mesh, PartitionSpec
        from jax.experimental.shard_map import shard_map
        from concourse import mybir
        from concourse.bass2jax import _bass_exec_p, install_neuronx_cc_hook

        install_neuronx_cc_hook()
        self.jax = jax

        in_names, out_names, out_avals, zero_outs = [], [], [], []
        for alloc in nc.m.functions[0].allocations:
            if not isinstance(alloc, mybir.MemoryLocationSet):
                continue
            name = alloc.memorylocations[0].name
            if alloc.kind == "ExternalInput":
                in_names.append(name)
            elif alloc.kind == "ExternalOutput":
                shape = tuple(alloc.tensor_shape)
                dtype = mybir.dt.np(alloc.dtype)
                out_names.append(name)
                out_avals.append(jax.core.ShapedArray(shape, dtype))
                zero_outs.append(np.zeros(shape, dtype))
        self.in_names = in_names
        self.out_names = out_names
        self.out_avals = out_avals
        self.zero_outs = zero_outs
        all_in_names = in_names + out_names

        def _body(*args):
            outs = _bass_exec_p.bind(
                *args,
                out_avals=tuple(out_avals),
                in_names=tuple(all_in_names),
                out_names=tuple(out_names),
                lowering_input_output_aliases=(),
                sim_require_finite=True,
                sim_require_nnan=True,
                nc=nc,
            )
            return tuple(outs)

        devices = jax.devices()[:N_CORES]
        assert len(devices) == N_CORES, f"need {N_CORES} cores, saw {len(devices)}"
        mesh = Mesh(np.asarray(devices), ("core",))
        n_args = len(in_names) + len(out_names)
        self.sharded = jax.jit(
            shard_map(
                _body,
                mesh=mesh,
                in_specs=(PartitionSpec("core"),) * n_args,
                out_specs=(PartitionSpec("core"),) * len(out_names),
                check_rep=False,
            ),
            keep_unused=True,
        )

    def run(self, in_maps):
        jax = self.jax
        args = [
            np.concatenate([np.asarray(m[name]) for m in in_maps], axis=0)
            for name in self.in_names
        ]
        args += [
            np.zeros((N_CORES * z.shape[0], *z.shape[1:]), z.dtype)
            for z in self.zero_outs
        ]
        outs = self.sharded(*args)
        jax.block_until_ready(outs)
        return [
            {
                name: np.asarray(outs[i]).reshape(
                    N_CORES, *self.out_avals[i].shape
                )[c]
                for i, name in enumerate(self.out_names)
            }
            for c in range(N_CORES)
        ]


def _host_pack(Q, K, V, mask, core):
    hpc = H
    flat = core * hpc
    b = flat // HFULL
    h0 = flat % HFULL

    q = np.ascontiguousarray(Q[b, h0 : h0 + hpc])
    k = np.ascontiguousarray(K[b, h0 : h0 + hpc])
    v = np.ascontiguousarray(V[b, h0 : h0 + hpc])
    m = mask[b, 0]

    qkt = np.stack([q.transpose(0, 2, 1), k.transpose(0, 2, 1)], axis=2).astype(
        ml_dtypes.bfloat16
    )

    vr = v.reshape(hpc, KB, 128, D)
    va = np.concatenate(
        [vr, np.ones((hpc, KB, 128, 1), np.float32)], axis=-1
    ).astype(ml_dtypes.bfloat16)

    mT = np.ascontiguousarray(m.T)
    mbias = np.where(mT, np.float32(MASK_NEG), np.float32(0.0)).astype(
        ml_dtypes.float8_e4m3
    )
    mbias = mbias.reshape(NG, G, 128, S).transpose(2, 0, 1, 3)

    return {
        "qkt": np.ascontiguousarray(qkt),
        "vaug": np.ascontiguousarray(va),
        "mb": np.ascontiguousarray(mbias),
    }


def _get_runner():
    if "runner" not in _STATE:
        _STATE["runner"] = _Runner(_build_program())
    return _STATE["runner"]


def kernel(Q, K, V, mask):
    Q = np.asarray(Q, dtype=np.float32)
    K = np.asarray(K, dtype=np.float32)
    V = np.asarray(V, dtype=np.float32)
    mask = np.asarray(mask).astype(bool)
    assert Q.shape == (B, HFULL, S, D), f"unexpected Q shape {Q.shape}"
    assert mask.shape == (B, 1, S, S), f"unexpected mask shape {mask.shape}"

    runner = _get_runner()
    in_maps = [_host_pack(Q, K, V, mask, c) for c in range(N_CORES)]
    results = runner.run(in_maps)

    out = np.empty((B, HFULL, S, D), np.float32)
    for core in range(N_CORES):
        flat = core * H
        b = flat // HFULL
        h0 = flat % HFULL
        # [H, NQC, 128, NJ, D] p-major -> [H, S, D]
        r = results[core]["out"].transpose(0, 1, 3, 2, 4).reshape(H, S, D)
        out[b, h0 : h0 + H] = r
    return out


# revision 3
# speedup vs baseline: 1.1560x; 1.1560x over previous
"""Masked multi-head attention on 8 Trainium2 NeuronCores (Bass/Tile).

Problem: Q,K,V [2, 16, 2048, 64] f32, mask [2, 1, 2048, 2048] bool ->
softmax(where(mask, -inf, QK^T) / sqrt(64)) @ V, computed as one SPMD Bass
program over 8 cores; each core owns 4 heads of one batch ((B,H) sharding).

Per-core kernel (per head, per 512-wide q-chunk), all bf16 matmul inputs:
  - scores^T[k, q] = K^T Q: bf16 matmuls, the D=64 contraction row-packed two
    k-blocks at a time into PE row groups (0,0)/(64,0).
  - mask: additive -240 bias in fp8e4m3 ([k, q]-transposed, host-prepared).
    One of 8 k-block groups per q-chunk gets it via a PE identity-matmul
    preload into PSUM (QK accumulates on top); the rest via DVE tensor_add in
    PSUM. This split balances DVE vs PE occupancy (measured, not modeled:
    more PE groups poison the QK accumulation pipeline).
  - exp on the Scalar engine with scale=1/8, PSUM -> SBUF, bf16 slab out.
    No max-subtraction: unmasked logits are O(5) and softmax is
    shift-invariant; masked logits sit at -30 after scaling (exp ~ 2e-11).
  - O^T = V_aug^T @ P^T bf16 matmuls accumulating over k in PSUM; V_aug has a
    ones column appended so row 64 of the accumulator is the softmax
    denominator for free.
  - Scalar-engine copy PSUM->SBUF (keeps DVE off the critical tail), PE
    transposes [65 x 128] tiles back to [q, d], one narrow DVE reciprocal
    (free dim 4 - wide reciprocals are ~7 cyc/elem) and one broadcasted DVE
    multiply normalize, DMA out (p-major layout, unshuffled on the host).
"""

import sys

sys.path.insert(0, "/opt/trn_rl_repo")

from contextlib import ExitStack

import numpy as np
import ml_dtypes

N_CORES = 8
B, HFULL, S, D = 2, 16, 2048, 64
H = (B * HFULL) // N_CORES  # heads per core
QC = 512
KB = S // 128
NQC = S // QC
NJ = QC // 128
G = 2  # k-blocks per PSUM scores tile
NG = KB // G
MASK_NEG = -240.0  # max finite in fp8e4m3 (IEEE variant); *1/8 -> -30 logit shift
MASK_ON_PE = 0.125  # fraction of k-block groups whose mask-add runs on PE

_STATE = {}


def _build_program():
    import concourse.bass as bass
    import concourse.tile as tile
    from concourse import bacc, mybir
    from concourse.masks import make_identity

    F32 = mybir.dt.float32
    BF16 = mybir.dt.bfloat16
    FP8 = mybir.dt.float8e4

    nc = bacc.Bacc(
        "TRN2", target_bir_lowering=False, debug=False, enable_partition_id=False
    )

    qkt = nc.dram_tensor("qkt", [H, 64, 2, S], BF16, kind="ExternalInput").ap()
    vaug = nc.dram_tensor("vaug", [H, KB, 128, D + 1], BF16, kind="ExternalInput").ap()
    mb = nc.dram_tensor("mb", [128, NG, G, S], FP8, kind="ExternalInput").ap()
    out = nc.dram_tensor("out", [H, NQC, 128, NJ, D], F32, kind="ExternalOutput").ap()

    n_pe_mask = int(round(NG * MASK_ON_PE))

    with tile.TileContext(nc) as tc, ExitStack() as ctx:
        const_pool = ctx.enter_context(tc.tile_pool(name="const", bufs=1))
        mb_pool = ctx.enter_context(tc.tile_pool(name="mbp", bufs=1))
        qk_pool = ctx.enter_context(tc.tile_pool(name="qkp", bufs=2))
        v_pool = ctx.enter_context(tc.tile_pool(name="vp", bufs=2))
        slab_pool = ctx.enter_context(tc.tile_pool(name="slab", bufs=2 * NG))
        o_pool = ctx.enter_context(tc.tile_pool(name="op", bufs=2))
        small_pool = ctx.enter_context(tc.tile_pool(name="smallp", bufs=8))
        ps_score_pool = ctx.enter_context(
            tc.tile_pool(name="psscore", bufs=3, space="PSUM")
        )
        ps_o_pool = ctx.enter_context(tc.tile_pool(name="pso", bufs=1, space="PSUM"))
        ps_t_pool = ctx.enter_context(tc.tile_pool(name="pst", bufs=1, space="PSUM"))

        ident_f = const_pool.tile([128, 128], F32)
        make_identity(nc, ident_f)
        ident_b = const_pool.tile([128, 128], FP8)
        make_identity(nc, ident_b)

        mb_t = mb_pool.tile([128, NG, G, S], FP8, tag="mb")
        hsplit = max(1, NG // 4)
        nc.sync.dma_start(mb_t[:, :hsplit], mb[:, :hsplit])
        nc.sync.dma_start(mb_t[:, hsplit:], mb[:, hsplit:])

        def emit_pv(pend):
            # PV + normalize for a q-chunk, emitted one chunk late so the PE
            # queue interleaves it with the NEXT chunk's QK matmuls instead of
            # stalling the Scalar engine's exp pipeline at the qc boundary.
            ph, pqc, pv_t, pslabs = pend
            ps_o = ps_o_pool.tile([D + 1, QC], F32, tag="pso")
            for kb in range(KB):
                nc.tensor.matmul(
                    ps_o[:],
                    pv_t[:, kb, :],
                    pslabs[kb // G][:, kb % G, :],
                    start=(kb == 0),
                    stop=(kb == KB - 1),
                )
            o_sb = o_pool.tile([D + 1, QC], F32, tag="osb")
            nc.scalar.copy(o_sb[:], ps_o[:])

            out_sb = o_pool.tile([128, NJ, D], F32, tag="outsb")
            ps_t = ps_t_pool.tile([128, NJ, D + 1], F32, tag="pst")
            for j in range(NJ):
                nc.tensor.transpose(
                    ps_t[:, j, :],
                    o_sb[:, bass.ts(j, 128)],
                    ident_f[: D + 1, : D + 1],
                )
            rcp = small_pool.tile([128, NJ], F32, tag="rcp")
            nc.vector.reciprocal(rcp[:], ps_t[:, :, D])
            nc.vector.tensor_mul(
                out_sb[:], ps_t[:, :, :D], rcp[:].broadcast_to((128, NJ, D))
            )
            nc.gpsimd.dma_start(out[ph, pqc], out_sb[:])

        pending = []
        for h in range(H):
            qk_t = qk_pool.tile([128, 2, S], BF16, tag="qk")
            nc.sync.dma_start(qk_t[:64, :, :], qkt[h])
            nc.sync.dma_start(qk_t[64:, :, :], qkt[h])
            qt_t = qk_t[:, 0, :]
            kt_t = qk_t[:, 1, :]
            v_t = v_pool.tile([128, KB, D + 1], BF16, tag="v")
            nc.gpsimd.dma_start(v_t[:], vaug[h].rearrange("kb p d -> p kb d"))

            for qc in range(NQC):
                qsl = bass.ts(qc, QC)
                slabs = []
                for g in range(NG):
                    ps = ps_score_pool.tile([128, G, QC], F32, tag="psscore")
                    on_pe = n_pe_mask > 0 and g % (NG // n_pe_mask) == 0
                    for i in range(G):
                        kb = G * g + i
                        half = kb % 2
                        lo, hi = half * 64, half * 64 + 64
                        psl = ps[:, i, :]
                        if on_pe:
                            nc.tensor.matmul(
                                psl,
                                ident_b[:],
                                mb_t[:, g, i, qsl],
                                start=True,
                                stop=False,
                            )
                        nc.tensor.matmul(
                            psl,
                            kt_t[lo:hi, bass.ts(kb, 128)],
                            qt_t[lo:hi, qsl],
                            start=not on_pe,
                            stop=True,
                        )
                    if not on_pe:
                        nc.vector.tensor_add(ps[:], ps[:], mb_t[:, g, :, qsl])
                    slab = slab_pool.tile([128, G, QC], BF16, tag="slab")
                    nc.scalar.activation(
                        slab[:], ps[:], mybir.ActivationFunctionType.Exp, scale=0.125
                    )
                    slabs.append(slab)

                pending.append((h, qc, v_t, slabs))
                if len(pending) > 1:
                    emit_pv(pending.pop(0))
        while pending:
            emit_pv(pending.pop(0))

    nc.compile()
    return nc


class _Runner:
    """shard_map jit over the 8 NeuronCores, reusable across calls."""

    def __init__(self, nc):
        import jax
        from jax.sharding import Mesh, PartitionSpec
        from jax.experimental.shard_map import shard_map
        from concourse import mybir
        from concourse.bass2jax import _bass_exec_p, install_neuronx_cc_hook

        install_neuronx_cc_hook()
        self.jax = jax

        in_names, out_names, out_avals, zero_outs = [], [], [], []
        for alloc in nc.m.functions[0].allocations:
            if not isinstance(alloc, mybir.MemoryLocationSet):
                continue
            name = alloc.memorylocations[0].name
            if alloc.kind == "ExternalInput":
                in_names.append(name)
            elif alloc.kind == "ExternalOutput":
                shape = tuple(alloc.tensor_shape)
                dtype = mybir.dt.np(alloc.dtype)
                out_names.append(name)
                out_avals.append(jax.core.ShapedArray(shape, dtype))
                zero_outs.append(np.zeros(shape, dtype))
        self.in_names = in_names
        self.out_names = out_names
        self.out_avals = out_avals
        self.zero_outs = zero_outs
        all_in_names = in_names + out_names

        def _body(*args):
            outs = _bass_exec_p.bind(
                *args,
                out_avals=tuple(out_avals),
                in_names=tuple(all_in_names),
                out_names=tuple(out_names),
                lowering_input_output_aliases=(),
                sim_require_finite=True,
                sim_require_nnan=True,
                nc=nc,
            )
            return tuple(outs)

        devices = jax.devices()[:N_CORES]
        assert len(devices) == N_CORES, f"need {N_CORES} cores, saw {len(devices)}"
        mesh = Mesh(np.asarray(devices), ("core",))
        n_args = len(in_names) + len(out_names)
        self.sharded = jax.jit(
            shard_map(
                _body,
                mesh=mesh,
                in_specs=(PartitionSpec("core"),) * n_args,
                out_specs=(PartitionSpec("core"),) * len(out_names),
                check_rep=False,
            ),
            keep_unused=True,
        )

    def run(self, in_maps):
        jax = self.jax
        args = [
            np.concatenate([np.asarray(m[name]) for m in in_maps], axis=0)
            for name in self.in_names
        ]
        args += [
            np.zeros((N_CORES * z.shape[0], *z.shape[1:]), z.dtype)
            for z in self.zero_outs
        ]
        outs = self.sharded(*args)
        jax.block_until_ready(outs)
        return [
            {
                name: np.asarray(outs[i]).reshape(
                    N_CORES, *self.out_avals[i].shape
                )[c]
                for i, name in enumerate(self.out_names)
            }
            for c in range(N_CORES)
        ]


def _host_pack(Q, K, V, mask, core):
    hpc = H
    flat = core * hpc
    b = flat // HFULL
    h0 = flat % HFULL

    q = np.ascontiguousarray(Q[b, h0 : h0 + hpc])
    k = np.ascontiguousarray(K[b, h0 : h0 + hpc])
    v = np.ascontiguousarray(V[b, h0 : h0 + hpc])
    m = mask[b, 0]

    qkt = np.stack([q.transpose(0, 2, 1), k.transpose(0, 2, 1)], axis=2).astype(
        ml_dtypes.bfloat16
    )

    vr = v.reshape(hpc, KB, 128, D)
    va = np.concatenate(
        [vr, np.ones((hpc, KB, 128, 1), np.float32)], axis=-1
    ).astype(ml_dtypes.bfloat16)

    mT = np.ascontiguousarray(m.T)
    mbias = np.where(mT, np.float32(MASK_NEG), np.float32(0.0)).astype(
        ml_dtypes.float8_e4m3
    )
    mbias = mbias.reshape(NG, G, 128, S).transpose(2, 0, 1, 3)

    return {
        "qkt": np.ascontiguousarray(qkt),
        "vaug": np.ascontiguousarray(va),
        "mb": np.ascontiguousarray(mbias),
    }


def _get_runner():
    if "runner" not in _STATE:
        _STATE["runner"] = _Runner(_build_program())
    return _STATE["runner"]


def kernel(Q, K, V, mask):
    Q = np.asarray(Q, dtype=np.float32)
    K = np.asarray(K, dtype=np.float32)
    V = np.asarray(V, dtype=np.float32)
    mask = np.asarray(mask).astype(bool)
    assert Q.shape == (B, HFULL, S, D), f"unexpected Q shape {Q.shape}"
    assert mask.shape == (B, 1, S, S), f"unexpected mask shape {mask.shape}"

    runner = _get_runner()
    in_maps = [_host_pack(Q, K, V, mask, c) for c in range(N_CORES)]
    results = runner.run(in_maps)

    out = np.empty((B, HFULL, S, D), np.float32)
    for core in range(N_CORES):
        flat = core * H
        b = flat // HFULL
        h0 = flat % HFULL
        # [H, NQC, 128, NJ, D] p-major -> [H, S, D]
        r = results[core]["out"].transpose(0, 1, 3, 2, 4).reshape(H, S, D)
        out[b, h0 : h0 + H] = r
    return out


# revision 4
# speedup vs baseline: 1.3300x; 1.1505x over previous
"""Masked multi-head attention on 8 Trainium2 NeuronCores (Bass/Tile).

Problem: Q,K,V [2, 16, 2048, 64] f32, mask [2, 1, 2048, 2048] bool ->
softmax(where(mask, -inf, QK^T) / sqrt(64)) @ V, computed as one SPMD Bass
program over 8 cores; each core owns 4 heads of one batch ((B,H) sharding).

Per-core kernel (per head, per 512-wide q-chunk), all bf16 matmul inputs:
  - scores^T[k, q] = K^T Q: bf16 matmuls, the D=64 contraction row-packed two
    k-blocks at a time into PE row groups (0,0)/(64,0).
  - exp on the Scalar engine with scale=1/8, PSUM -> SBUF, bf16 slab out,
    issued straight after QK (no PSUM mask pass keeps ACT un-stalled). No
    max-subtraction: unmasked logits are O(5) and softmax is shift-invariant.
  - mask applied POST-exp as a multiplicative 0/1 bf16 mask on the slab:
    exp(s/8)*m == exp(s/8 + mask_bias). The all-bf16 SBUF->SBUF tensor_mul
    hits DVE's 2x_1P packed mode (~690ns vs ~1224ns for the f32-PSUM additive
    form) and the mask zeros are exact.
  - O^T = V_aug^T @ P^T bf16 matmuls accumulating over k in PSUM; V_aug has a
    ones column appended so row 64 of the accumulator is the softmax
    denominator for free.
  - DVE copy PSUM->SBUF (ACT is the busiest engine in this variant), PE
    transposes [65 x 128] tiles back to [q, d], one narrow DVE reciprocal
    (free dim 4 - wide reciprocals are ~7 cyc/elem) and one broadcasted DVE
    multiply normalize, DMA out (p-major layout, unshuffled on the host).
"""

import sys

sys.path.insert(0, "/opt/trn_rl_repo")

from contextlib import ExitStack

import numpy as np
import ml_dtypes

N_CORES = 8
B, HFULL, S, D = 2, 16, 2048, 64
H = (B * HFULL) // N_CORES  # heads per core
QC = 512
KB = S // 128
NQC = S // QC
NJ = QC // 128
G = 2  # k-blocks per PSUM scores tile
NG = KB // G

_STATE = {}


def _build_program():
    import concourse.bass as bass
    import concourse.tile as tile
    from concourse import bacc, mybir
    from concourse.masks import make_identity

    F32 = mybir.dt.float32
    BF16 = mybir.dt.bfloat16

    nc = bacc.Bacc(
        "TRN2", target_bir_lowering=False, debug=False, enable_partition_id=False
    )

    qkt = nc.dram_tensor("qkt", [H, 64, 2, S], BF16, kind="ExternalInput").ap()
    vaug = nc.dram_tensor("vaug", [H, KB, 128, D + 1], BF16, kind="ExternalInput").ap()
    mb = nc.dram_tensor("mb", [128, NG, G, S], BF16, kind="ExternalInput").ap()
    out = nc.dram_tensor("out", [H, NQC, 128, NJ, D], F32, kind="ExternalOutput").ap()

    with tile.TileContext(nc) as tc, ExitStack() as ctx:
        const_pool = ctx.enter_context(tc.tile_pool(name="const", bufs=1))
        mb_pool = ctx.enter_context(tc.tile_pool(name="mbp", bufs=1))
        qk_pool = ctx.enter_context(tc.tile_pool(name="qkp", bufs=2))
        v_pool = ctx.enter_context(tc.tile_pool(name="vp", bufs=2))
        slab_pool = ctx.enter_context(tc.tile_pool(name="slab", bufs=2 * NG))
        o_pool = ctx.enter_context(tc.tile_pool(name="op", bufs=2))
        small_pool = ctx.enter_context(tc.tile_pool(name="smallp", bufs=8))
        ps_score_pool = ctx.enter_context(
            tc.tile_pool(name="psscore", bufs=3, space="PSUM")
        )
        ps_o_pool = ctx.enter_context(tc.tile_pool(name="pso", bufs=1, space="PSUM"))
        ps_t_pool = ctx.enter_context(tc.tile_pool(name="pst", bufs=1, space="PSUM"))

        ident_f = const_pool.tile([128, 128], F32)
        make_identity(nc, ident_f)

        mb_t = mb_pool.tile([128, NG, G, S], BF16, tag="mb")
        hsplit = max(1, NG // 4)
        nc.sync.dma_start(mb_t[:, :hsplit], mb[:, :hsplit])
        nc.sync.dma_start(mb_t[:, hsplit:], mb[:, hsplit:])

        def emit_pv(pend):
            # PV + normalize for a q-chunk, emitted one chunk late so the PE
            # queue interleaves it with the NEXT chunk's QK matmuls instead of
            # stalling the Scalar engine's exp pipeline at the qc boundary.
            ph, pqc, pv_t, pslabs = pend
            ps_o = ps_o_pool.tile([D + 1, QC], F32, tag="pso")
            for kb in range(KB):
                nc.tensor.matmul(
                    ps_o[:],
                    pv_t[:, kb, :],
                    pslabs[kb // G][:, kb % G, :],
                    start=(kb == 0),
                    stop=(kb == KB - 1),
                )
            o_sb = o_pool.tile([D + 1, QC], F32, tag="osb")
            nc.vector.tensor_copy(o_sb[:], ps_o[:])

            out_sb = o_pool.tile([128, NJ, D], F32, tag="outsb")
            ps_t = ps_t_pool.tile([128, NJ, D + 1], F32, tag="pst")
            for j in range(NJ):
                nc.tensor.transpose(
                    ps_t[:, j, :],
                    o_sb[:, bass.ts(j, 128)],
                    ident_f[: D + 1, : D + 1],
                )
            rcp = small_pool.tile([128, NJ], F32, tag="rcp")
            nc.vector.reciprocal(rcp[:], ps_t[:, :, D])
            nc.vector.tensor_mul(
                out_sb[:], ps_t[:, :, :D], rcp[:].broadcast_to((128, NJ, D))
            )
            nc.gpsimd.dma_start(out[ph, pqc], out_sb[:])

        pending = []
        for h in range(H):
            qk_t = qk_pool.tile([128, 2, S], BF16, tag="qk")
            nc.sync.dma_start(qk_t[:64, :, :], qkt[h])
            nc.sync.dma_start(qk_t[64:, :, :], qkt[h])
            qt_t = qk_t[:, 0, :]
            kt_t = qk_t[:, 1, :]
            v_t = v_pool.tile([128, KB, D + 1], BF16, tag="v")
            nc.gpsimd.dma_start(v_t[:], vaug[h].rearrange("kb p d -> p kb d"))

            for qc in range(NQC):
                qsl = bass.ts(qc, QC)
                slabs = []
                for g in range(NG):
                    ps = ps_score_pool.tile([128, G, QC], F32, tag="psscore")
                    for i in range(G):
                        kb = G * g + i
                        half = kb % 2
                        lo, hi = half * 64, half * 64 + 64
                        nc.tensor.matmul(
                            ps[:, i, :],
                            kt_t[lo:hi, bass.ts(kb, 128)],
                            qt_t[lo:hi, qsl],
                            start=True,
                            stop=True,
                        )
                    slab = slab_pool.tile([128, G, QC], BF16, tag="slab")
                    nc.scalar.activation(
                        slab[:], ps[:], mybir.ActivationFunctionType.Exp, scale=0.125
                    )
                    nc.vector.tensor_mul(slab[:], slab[:], mb_t[:, g, :, qsl])
                    slabs.append(slab)

                pending.append((h, qc, v_t, slabs))
                if len(pending) > 1:
                    emit_pv(pending.pop(0))
        while pending:
            emit_pv(pending.pop(0))

    nc.compile()
    return nc


class _Runner:
    """shard_map jit over the 8 NeuronCores, reusable across calls."""

    def __init__(self, nc):
        import jax
        from jax.sharding import Mesh, PartitionSpec
        from jax.experimental.shard_map import shard_map
        from concourse import mybir
        from concourse.bass2jax import _bass_exec_p, install_neuronx_cc_hook

        install_neuronx_cc_hook()
        self.jax = jax

        in_names, out_names, out_avals, zero_outs = [], [], [], []
        for alloc in nc.m.functions[0].allocations:
            if not isinstance(alloc, mybir.MemoryLocationSet):
                continue
            name = alloc.memorylocations[0].name
            if alloc.kind == "ExternalInput":
                in_names.append(name)
            elif alloc.kind == "ExternalOutput":
                shape = tuple(alloc.tensor_shape)
                dtype = mybir.dt.np(alloc.dtype)
                out_names.append(name)
                out_avals.append(jax.core.ShapedArray(shape, dtype))
                zero_outs.append(np.zeros(shape, dtype))
        self.in_names = in_names
        self.out_names = out_names
        self.out_avals = out_avals
        self.zero_outs = zero_outs
        all_in_names = in_names + out_names

        def _body(*args):
            outs = _bass_exec_p.bind(
                *args,
                out_avals=tuple(out_avals),
                in_names=tuple(all_in_names),
                out_names=tuple(out_names),
                lowering_input_output_aliases=(),
                sim_require_finite=True,
                sim_require_nnan=True,
                nc=nc,
            )
            return tuple(outs)

        devices = jax.devices()[:N_CORES]
        assert len(devices) == N_CORES, f"need {N_CORES} cores, saw {len(devices)}"
        mesh = Mesh(np.asarray(devices), ("core",))
        n_args = len(in_names) + len(out_names)
        self.sharded = jax.jit(
            shard_map(
                _body,
                mesh=mesh,
                in_specs=(PartitionSpec("core"),) * n_args,
                out_specs=(PartitionSpec("core"),) * len(out_names),
                check_rep=False,
            ),
            keep_unused=True,
        )

    def run(self, in_maps):
        jax = self.jax
        args = [
            np.concatenate([np.asarray(m[name]) for m in in_maps], axis=0)
            for name in self.in_names
        ]
        args += [
            np.zeros((N_CORES * z.shape[0], *z.shape[1:]), z.dtype)
            for z in self.zero_outs
        ]
        outs = self.sharded(*args)
        jax.block_until_ready(outs)
        return [
            {
                name: np.asarray(outs[i]).reshape(
                    N_CORES, *self.out_avals[i].shape
                )[c]
                for i, name in enumerate(self.out_names)
            }
            for c in range(N_CORES)
        ]


def _host_pack(Q, K, V, mask, core):
    hpc = H
    flat = core * hpc
    b = flat // HFULL
    h0 = flat % HFULL

    q = np.ascontiguousarray(Q[b, h0 : h0 + hpc])
    k = np.ascontiguousarray(K[b, h0 : h0 + hpc])
    v = np.ascontiguousarray(V[b, h0 : h0 + hpc])
    m = mask[b, 0]

    qkt = np.stack([q.transpose(0, 2, 1), k.transpose(0, 2, 1)], axis=2).astype(
        ml_dtypes.bfloat16
    )

    vr = v.reshape(hpc, KB, 128, D)
    va = np.concatenate(
        [vr, np.ones((hpc, KB, 128, 1), np.float32)], axis=-1
    ).astype(ml_dtypes.bfloat16)

    mT = np.ascontiguousarray(m.T)
    mbias = np.where(mT, np.float32(0.0), np.float32(1.0)).astype(
        ml_dtypes.bfloat16
    )
    mbias = mbias.reshape(NG, G, 128, S).transpose(2, 0, 1, 3)

    return {
        "qkt": np.ascontiguousarray(qkt),
        "vaug": np.ascontiguousarray(va),
        "mb": np.ascontiguousarray(mbias),
    }


def _get_runner():
    if "runner" not in _STATE:
        _STATE["runner"] = _Runner(_build_program())
    return _STATE["runner"]


def kernel(Q, K, V, mask):
    Q = np.asarray(Q, dtype=np.float32)
    K = np.asarray(K, dtype=np.float32)
    V = np.asarray(V, dtype=np.float32)
    mask = np.asarray(mask).astype(bool)
    assert Q.shape == (B, HFULL, S, D), f"unexpected Q shape {Q.shape}"
    assert mask.shape == (B, 1, S, S), f"unexpected mask shape {mask.shape}"

    runner = _get_runner()
    in_maps = [_host_pack(Q, K, V, mask, c) for c in range(N_CORES)]
    results = runner.run(in_maps)

    out = np.empty((B, HFULL, S, D), np.float32)
    for core in range(N_CORES):
        flat = core * H
        b = flat // HFULL
        h0 = flat % HFULL
        # [H, NQC, 128, NJ, D] p-major -> [H, S, D]
        r = results[core]["out"].transpose(0, 1, 3, 2, 4).reshape(H, S, D)
        out[b, h0 : h0 + H] = r
    return out
